# revision 1
# baseline (speedup 1.0000x reference)
"""Causal multi-head self-attention on 8 TRN2 NeuronCores (Bass/Tile).

Problem (hardcoded): x[2, 2048, 1024], Wq/Wk/Wv/Wo [1024, 1024] (nn.Linear
convention, out x in), H=16 heads, dk=64, causal softmax, y = attn @ Wo.T.

Sharding: 2-way data parallel (batch) x 4-way tensor parallel (head groups of
4). Each core computes q/k/v projections for its 4 heads, causal attention,
and a partial output projection against its 256-column slice of Wo. The host
sums the 4 partial [2048, 1024] outputs per batch (the "all-reduce").

Device kernel design notes:
  - Everything runs in "transposed" orientation so no on-device transposes are
    needed: QT/KT [256, S] = W @ x^T, V [S, 256] = x @ Wv^T, scoresT[j, i] per
    head, PV output [64, i], final y [i, o] (natural).
  - fp32r (TF32-like, 1 cycle/row for moving dim >= 256) for all matmuls;
    measured ~1e-4 matmul rel err.
  - Causal: tiles with j > i skipped entirely; diagonal-crossing [128, 512]
    tiles compute only columns >= 128*r and apply a [128, 128] triangular
     0/1 mask after exp. Softmax needs no max subtraction (|scores| <~ 7
    for this problem's N(0,1)-scale data; exp is safe in fp32).
  - Softmax denominator = ones[128, 64].T @ pT accumulated on the tensor
    engine, which also broadcasts it across 64 partitions for free; then
    reciprocal (DVE) and a fused multiply during the PV PSUM->SBUF move.
"""

import os
import numpy as np

import concourse.mybir as mybir
import concourse.tile as tile
from concourse import bacc
from concourse import bass_utils

F32 = mybir.dt.float32
F32R = mybir.dt.float32r
EXP = mybir.ActivationFunctionType.Exp
MULT = mybir.AluOpType.mult

P = 128        # partitions
F = 512        # free-dim chunk (fp32 max moving dim / one PSUM bank)
D = 1024       # model dim
E = 256        # per-core head-group width (4 heads x 64)
DK = 64        # head dim
HL = 4         # heads per core
NK = D // P    # contraction k-tiles for projections

LAST_RESULTS = None  # test harness can inspect exec_time_ns etc.


def build(S: int = 2048):
    """Build the per-core Bass program (same program on all 8 cores)."""
    NIC = S // F     # i-chunks
    NJT = S // P     # j-tiles

    nc = bacc.Bacc("TRN2", target_bir_lowering=False, debug=False,
                   enable_asserts=False)
    xT_d = nc.dram_tensor("xT", [D, S], F32, kind="ExternalInput").ap()
    wqT_d = nc.dram_tensor("wqT", [D, E], F32, kind="ExternalInput").ap()
    wkT_d = nc.dram_tensor("wkT", [D, E], F32, kind="ExternalInput").ap()
    wvT_d = nc.dram_tensor("wvT", [D, E], F32, kind="ExternalInput").ap()
    woT_d = nc.dram_tensor("woT", [E, D], F32, kind="ExternalInput").ap()
    tri_d = nc.dram_tensor("tri", [P, P], F32, kind="ExternalInput").ap()
    ones_d = nc.dram_tensor("ones", [P, DK], F32, kind="ExternalInput").ap()
    y_d = nc.dram_tensor("y", [S, D], F32, kind="ExternalOutput").ap()

    with tile.TileContext(nc) as tc:
        with tc.tile_pool(name="persist", bufs=1) as pp:
            tri_sb = pp.tile([P, P], F32)
            nc.sync.dma_start(tri_sb[:], tri_d)
            ones_sb = pp.tile([P, DK], F32R)
            nc.sync.dma_start(ones_sb[:], ones_d.bitcast(F32R))
            wo_sb = pp.tile([P, E // P, D], F32R)
            nc.sync.dma_start(wo_sb[:],
                              woT_d.rearrange("(kt p) o -> p kt o", p=P).bitcast(F32R))
            qT_sb = pp.tile([P, E // P, S], F32R)
            kT_sb = pp.tile([P, E // P, S], F32R)
            v_sb = pp.tile([P, NJT, E], F32R)
            outT_sb = pp.tile([P, E // P, S], F32R)

            # ---- Phase A: projections (QT, KT, V) ----
            with (
                tc.tile_pool(name="phA", bufs=1) as pa,
                tc.tile_pool(name="psA", bufs=4, space="PSUM") as psA,
            ):
                wq_sb = pa.tile([P, NK, E], F32R, tag="w")
                wk_sb = pa.tile([P, NK, E], F32R, tag="w")
                wv_sb = pa.tile([P, NK, E], F32R, tag="w")
                nc.sync.dma_start(wq_sb[:],
                                  wqT_d.rearrange("(kt p) e -> p kt e", p=P).bitcast(F32R))
                nc.sync.dma_start(wk_sb[:],
                                  wkT_d.rearrange("(kt p) e -> p kt e", p=P).bitcast(F32R))
                nc.sync.dma_start(wv_sb[:],
                                  wvT_d.rearrange("(kt p) e -> p kt e", p=P).bitcast(F32R))
                x_sb = pa.tile([P, NK, S], F32R)
                xT_r = xT_d.rearrange("(kt p) s -> p kt s", p=P).bitcast(F32R)
                for k in range(NK):
                    nc.sync.dma_start(x_sb[:, k], xT_r[:, k])

                # QT / KT: [E, S] = (wT k-tile).T @ (xT k-tile), accumulated over k
                for dst, w in ((qT_sb, wq_sb), (kT_sb, wk_sb)):
                    for et in range(E // P):
                        for ic in range(NIC):
                            ps = psA.tile([P, F], F32, tag="psA")
                            for k in range(NK):
                                nc.tensor.matmul(
                                    ps,
                                    lhsT=w[:, k, et * P:(et + 1) * P],
                                    rhs=x_sb[:, k, ic * F:(ic + 1) * F],
                                    start=(k == 0), stop=(k == NK - 1),
                                )
                            nc.any.tensor_copy(dst[:, et, ic * F:(ic + 1) * F], ps)
                # V: [S, E] = (xT k-tile).T @ wvT, accumulated over k
                for jt in range(NJT):
                    ps = psA.tile([P, F], F32, tag="psA")
                    for k in range(NK):
                        nc.tensor.matmul(
                            ps[:, :E],
                            lhsT=x_sb[:, k, jt * P:(jt + 1) * P],
                            rhs=wv_sb[:, k],
                            start=(k == 0), stop=(k == NK - 1),
                        )
                    nc.any.tensor_copy(v_sb[:, jt], ps[:, :E])

            # ---- Phase B: attention + output projection ----
            with (
                tc.tile_pool(name="pT", bufs=min(4 * NIC + 8, 28)) as ptp,
                tc.tile_pool(name="rd", bufs=3) as rdp,
                tc.tile_pool(name="ysb", bufs=4) as yp,
                tc.tile_pool(name="ps_s", bufs=3, space="PSUM") as pss,
                tc.tile_pool(name="ps_d", bufs=2, space="PSUM") as psd,
                tc.tile_pool(name="ps_pv", bufs=2, space="PSUM") as pspv,
                tc.tile_pool(name="ps_y", bufs=1, space="PSUM") as psy,
            ):
                def emit_wo(ic):
                    # y[i-tile, :] = outT.T @ woT for the 4 i-tiles of chunk ic
                    for it in range(ic * (F // P), (ic + 1) * (F // P)):
                        for oc in range(D // F):
                            ps = psy.tile([P, F], F32, tag="psy")
                            for et in range(E // P):
                                nc.tensor.matmul(
                                    ps,
                                    lhsT=outT_sb[:, et, it * P:(it + 1) * P],
                                    rhs=wo_sb[:, et, oc * F:(oc + 1) * F],
                                    start=(et == 0), stop=(et == E // P - 1),
                                )
                            yt = yp.tile([P, F], F32, tag="y")
                            nc.any.tensor_copy(yt[:], ps)
                            nc.sync.dma_start(
                                y_d[it * P:(it + 1) * P, oc * F:(oc + 1) * F], yt[:])

                for ic in range(NIC):
                    for h in range(HL):
                        et = h // 2
                        bp = (h % 2) * DK
                        njt = (ic + 1) * (F // P)
                        # scores (transposed) + exp + causal mask
                        ptiles = []
                        for jt in range(njt):
                            r = jt - ic * (F // P)
                            col0 = max(0, r * P)
                            ps = pss.tile([P, F], F32, tag="pss")
                            nc.tensor.matmul(
                                ps[:, col0:],
                                lhsT=kT_sb[bp:bp + DK, et, jt * P:(jt + 1) * P],
                                rhs=qT_sb[bp:bp + DK, et, ic * F + col0:(ic + 1) * F],
                                start=True, stop=True,
                            )
                            pt = ptp.tile([P, F], F32R, tag="pt")
                            nc.scalar.activation(pt[:, col0:], ps[:, col0:], EXP)
                            if r >= 0:
                                nc.vector.tensor_tensor(
                                    pt[:, col0:col0 + P], pt[:, col0:col0 + P],
                                    tri_sb[:], MULT)
                            ptiles.append((pt, col0))
                        # denominator (broadcast over 64 partitions by the PE)
                        ps_den = psd.tile([DK, F], F32, tag="psd")
                        for idx, (pt, col0) in enumerate(ptiles):
                            nc.tensor.matmul(
                                ps_den[:, col0:], lhsT=ones_sb[:], rhs=pt[:, col0:],
                                start=(idx == 0), stop=(idx == njt - 1),
                            )
                        # PV
                        ps_o = pspv.tile([DK, F], F32, tag="pspv")
                        for idx, (pt, col0) in enumerate(ptiles):
                            nc.tensor.matmul(
                                ps_o[:, col0:],
                                lhsT=v_sb[:, idx, h * DK:(h + 1) * DK],
                                rhs=pt[:, col0:],
                                start=(idx == 0), stop=(idx == njt - 1),
                            )
                        # normalize: outT = ps_o * (1 / denom)
                        rd = rdp.tile([DK, F], F32, tag="rd")
                        nc.vector.reciprocal(rd[:], ps_den[:])
                        nc.vector.tensor_tensor(
                            outT_sb[bp:bp + DK, et, ic * F:(ic + 1) * F],
                            ps_o[:], rd[:], MULT)
                    if ic >= 1:
                        emit_wo(ic - 1)
                emit_wo(NIC - 1)

    nc.compile()
    return nc


_CACHE = {}


def _get_nc(S):
    if S not in _CACHE:
        _CACHE[S] = build(S)
    return _CACHE[S]


def kernel(x, Wq, Wk, Wv, Wo):
    global LAST_RESULTS
    x = np.asarray(x, dtype=np.float32)
    Wq = np.asarray(Wq, dtype=np.float32)
    Wk = np.asarray(Wk, dtype=np.float32)
    Wv = np.asarray(Wv, dtype=np.float32)
    Wo = np.asarray(Wo, dtype=np.float32)
    B, S, D_ = x.shape
    nc = _get_nc(S)

    tri = np.triu(np.ones((P, P), np.float32))          # keep j' <= t
    ones = np.ones((P, DK), np.float32)
    scale = np.float32(1.0 / np.sqrt(DK))

    in_maps = []
    for c in range(8):
        b, g = divmod(c, 4)
        sl = slice(E * g, E * (g + 1))
        in_maps.append({
            "xT": np.ascontiguousarray(x[b].T),
            "wqT": np.ascontiguousarray((Wq[sl] * scale).T),
            "wkT": np.ascontiguousarray(Wk[sl].T),
            "wvT": np.ascontiguousarray(Wv[sl].T),
            "woT": np.ascontiguousarray(Wo[:, sl].T),
            "tri": tri,
            "ones": ones,
        })

    res = bass_utils.run_bass_kernel_spmd(
        nc, in_maps, core_ids=list(range(8)),
        trace=bool(os.environ.get("KERNEL_TRACE")),
    )
    LAST_RESULTS = res

    y = np.zeros((B, S, D_), np.float32)
    for c in range(8):
        y[c // 4] += res.results[c]["y"]
    return y


if __name__ == "__main__":
    # small-S self test against numpy
    S = 512
    rng = np.random.default_rng(0)
    B, H, dk = 2, 16, 64
    x = rng.standard_normal((B, S, D)).astype(np.float32)
    sc = 1.0 / np.sqrt(D)
    Wq = (rng.standard_normal((D, D)) * sc).astype(np.float32)
    Wk = (rng.standard_normal((D, D)) * sc).astype(np.float32)
    Wv = (rng.standard_normal((D, D)) * sc).astype(np.float32)
    Wo = (rng.standard_normal((D, D)) * sc).astype(np.float32)

    def ref(x, Wq, Wk, Wv, Wo):
        x64 = x.astype(np.float64)
        q = (x64 @ Wq.T.astype(np.float64)).reshape(B, S, H, dk).transpose(0, 2, 1, 3)
        k = (x64 @ Wk.T.astype(np.float64)).reshape(B, S, H, dk).transpose(0, 2, 1, 3)
        v = (x64 @ Wv.T.astype(np.float64)).reshape(B, S, H, dk).transpose(0, 2, 1, 3)
        s = np.einsum("bhid,bhjd->bhij", q, k) / np.sqrt(dk)
        mask = np.triu(np.ones((S, S), bool), k=1)
        s = np.where(mask, -np.inf, s)
        s -= s.max(axis=-1, keepdims=True)
        p = np.exp(s)
        p /= p.sum(axis=-1, keepdims=True)
        o = np.einsum("bhij,bhjd->bhid", p, v).transpose(0, 2, 1, 3).reshape(B, S, D)
        return o @ Wo.T.astype(np.float64)

    expected = ref(x, Wq, Wk, Wv, Wo)
    actual = kernel(x, Wq, Wk, Wv, Wo)
    err = np.abs(actual - expected).max() / np.abs(expected).max()
    print("self-test S=512 max rel err:", err)
    assert err < 2e-3, err
    print("PASS")


# revision 3
# speedup vs baseline: 1.1105x; 1.1105x over previous
"""Causal multi-head self-attention on 8 TRN2 NeuronCores (Bass/Tile).

Problem (hardcoded): x[2, 2048, 1024], Wq/Wk/Wv/Wo [1024, 1024] (nn.Linear
convention, out x in), H=16 heads, dk=64, causal softmax, y = attn @ Wo.T.

Sharding: 2-way data parallel (batch) x 4-way tensor parallel (head groups of
4). Each core computes q/k/v projections for its 4 heads, causal attention,
and a partial output projection against its 256-column slice of Wo. The host
sums the 4 partial [2048, 1024] outputs per batch (the "all-reduce").

Device kernel design notes:
  - Everything runs in "transposed" orientation so no on-device transposes are
    needed: QT/KT [256, S] = W @ x^T, V [S, 256] = x @ Wv^T, scoresT[j, i] per
    head, PV output [64+1, i], final y [i, o] (natural).
  - fp32r (TF32-like, 1 cycle/row for moving dim >= 256) for all matmuls;
    measured ~1e-4 matmul rel err.
  - Causal: tiles with j > i skipped entirely; diagonal-crossing [128, 512]
    tiles compute only columns >= 128*r and apply a [128, 128] triangular
    0/1 mask after exp. Softmax needs no max subtraction (|scores| <~ 7
    for this problem's N(0,1)-scale data; exp is safe in fp32).
  - Softmax denominator comes free from the PV matmul: V is augmented with a
    ones column (lhsT [j, 65]), so PSUM row 64 = sum_j p[j, i]. It is
    broadcast across 64 partitions with a K=1 outer-product matmul, inverted
    with one fast-reciprocal DVE op, and applied during the PV PSUM->SBUF
    move.
  - Emission is head-pipelined (scores of head h+1 are issued before PV of
    head h) so the tensor engine never waits on ScalarE's exp stream.
"""

import os
import numpy as np

import concourse.mybir as mybir
import concourse.tile as tile
from concourse import bacc
from concourse import bass_utils

F32 = mybir.dt.float32
F32R = mybir.dt.float32r
EXP = mybir.ActivationFunctionType.Exp
MULT = mybir.AluOpType.mult

P = 128        # partitions
F = 512        # free-dim chunk (fp32 max moving dim / one PSUM bank)
D = 1024       # model dim
E = 256        # per-core head-group width (4 heads x 64)
DK = 64        # head dim
HL = 4         # heads per core
NK = D // P    # contraction k-tiles for projections

LAST_RESULTS = None  # test harness can inspect exec_time_ns etc.


def build(S: int = 2048):
    """Build the per-core Bass program (same program on all 8 cores)."""
    NIC = S // F     # i-chunks
    NJT = S // P     # j-tiles
    TPC = F // P     # j-tiles per i-chunk (4)

    nc = bacc.Bacc("TRN2", target_bir_lowering=False, debug=False,
                   enable_asserts=False)
    xT_d = nc.dram_tensor("xT", [D, S], F32, kind="ExternalInput").ap()
    wqT_d = nc.dram_tensor("wqT", [D, E], F32, kind="ExternalInput").ap()
    wkT_d = nc.dram_tensor("wkT", [D, E], F32, kind="ExternalInput").ap()
    wvT_d = nc.dram_tensor("wvT", [D, E], F32, kind="ExternalInput").ap()
    woT_d = nc.dram_tensor("woT", [E, D], F32, kind="ExternalInput").ap()
    tri_d = nc.dram_tensor("tri", [P, P], F32, kind="ExternalInput").ap()
    ones_d = nc.dram_tensor("ones", [P, DK], F32, kind="ExternalInput").ap()
    y_d = nc.dram_tensor("y", [S, D], F32, kind="ExternalOutput").ap()

    with tile.TileContext(nc) as tc:
        with tc.tile_pool(name="persist", bufs=1) as pp:
            tri_sb = pp.tile([P, P], F32)
            nc.sync.dma_start(tri_sb[:], tri_d)
            ones_sb = pp.tile([P, DK], F32R)
            nc.sync.dma_start(ones_sb[:], ones_d.bitcast(F32R))
            wo_sb = pp.tile([P, E // P, D], F32R)
            nc.sync.dma_start(wo_sb[:],
                              woT_d.rearrange("(kt p) o -> p kt o", p=P).bitcast(F32R))
            qT_sb = pp.tile([P, E // P, S], F32R)
            kT_sb = pp.tile([P, E // P, S], F32R)
            v_sb = pp.tile([P, NJT, HL, DK + 1], F32R)
            outT_sb = pp.tile([P, E // P, S], F32R)

            # ---- Phase A: projections (QT, KT, V) ----
            with (
                tc.tile_pool(name="phA", bufs=1) as pa,
                tc.tile_pool(name="psA", bufs=4, space="PSUM") as psA,
            ):
                wq_sb = pa.tile([P, NK, E], F32R, tag="w")
                wk_sb = pa.tile([P, NK, E], F32R, tag="w")
                wv_sb = pa.tile([P, NK, E], F32R, tag="w")
                nc.sync.dma_start(wq_sb[:],
                                  wqT_d.rearrange("(kt p) e -> p kt e", p=P).bitcast(F32R))
                nc.sync.dma_start(wk_sb[:],
                                  wkT_d.rearrange("(kt p) e -> p kt e", p=P).bitcast(F32R))
                nc.sync.dma_start(wv_sb[:],
                                  wvT_d.rearrange("(kt p) e -> p kt e", p=P).bitcast(F32R))
                x_sb = pa.tile([P, NK, S], F32R)
                xT_r = xT_d.rearrange("(kt p) s -> p kt s", p=P).bitcast(F32R)
                for k in range(NK):
                    nc.sync.dma_start(x_sb[:, k], xT_r[:, k])

                # ones column of the augmented V (all j-tiles at once)
                nc.vector.tensor_copy(
                    v_sb[:, :, :, DK].rearrange("p a b -> p (a b)"),
                    ones_sb[:, :1].to_broadcast([P, NJT * HL]))

                # QT / KT: [E, S] = (wT k-tile).T @ (xT k-tile), accumulated over k
                for dst, w in ((qT_sb, wq_sb), (kT_sb, wk_sb)):
                    for et in range(E // P):
                        for ic in range(NIC):
                            ps = psA.tile([P, F], F32, tag="psA")
                            for k in range(NK):
                                nc.tensor.matmul(
                                    ps,
                                    lhsT=w[:, k, et * P:(et + 1) * P],
                                    rhs=x_sb[:, k, ic * F:(ic + 1) * F],
                                    start=(k == 0), stop=(k == NK - 1),
                                )
                            nc.any.tensor_copy(dst[:, et, ic * F:(ic + 1) * F], ps)
                # V: [S, E] = (xT k-tile).T @ wvT, accumulated over k
                for jt in range(NJT):
                    ps = psA.tile([P, F], F32, tag="psA")
                    for k in range(NK):
                        nc.tensor.matmul(
                            ps[:, :E],
                            lhsT=x_sb[:, k, jt * P:(jt + 1) * P],
                            rhs=wv_sb[:, k],
                            start=(k == 0), stop=(k == NK - 1),
                        )
                    nc.any.tensor_copy(
                        v_sb[:, jt, :, :DK],
                        ps[:, :E].rearrange("p (h d) -> p h d", h=HL))

            # ---- Phase B: attention + output projection ----
            with (
                tc.tile_pool(name="pT", bufs=min(8 * TPC + 6, 38)) as ptp,
                tc.tile_pool(name="den", bufs=3) as denp,
                tc.tile_pool(name="rcp", bufs=3) as rcpp,
                tc.tile_pool(name="ysb", bufs=4) as yp,
                tc.tile_pool(name="ps_s", bufs=3, space="PSUM") as pss,
                tc.tile_pool(name="ps_pv", bufs=3, space="PSUM") as pspv,
                tc.tile_pool(name="ps_b", bufs=1, space="PSUM") as psb,
                tc.tile_pool(name="ps_y", bufs=1, space="PSUM") as psy,
            ):
                def emit_wo(ic):
                    # y[i-tile, :] = outT.T @ woT for the i-tiles of chunk ic
                    for it in range(ic * TPC, (ic + 1) * TPC):
                        for oc in range(D // F):
                            ps = psy.tile([P, F], F32, tag="psy")
                            for et in range(E // P):
                                nc.tensor.matmul(
                                    ps,
                                    lhsT=outT_sb[:, et, it * P:(it + 1) * P],
                                    rhs=wo_sb[:, et, oc * F:(oc + 1) * F],
                                    start=(et == 0), stop=(et == E // P - 1),
                                )
                            yt = yp.tile([P, F], F32, tag="y")
                            nc.vector.tensor_copy(yt[:], ps)
                            nc.sync.dma_start(
                                y_d[it * P:(it + 1) * P, oc * F:(oc + 1) * F], yt[:])

                def s_stream(h, ic):
                    """scores (transposed) + exp + causal mask for one head/chunk."""
                    et = h // 2
                    bp = (h % 2) * DK
                    njt = (ic + 1) * TPC
                    ptiles = []
                    for jt in range(njt):
                        r = jt - ic * TPC
                        col0 = max(0, r * P)
                        ps = pss.tile([P, F], F32, tag="pss")
                        nc.tensor.matmul(
                            ps[:, col0:],
                            lhsT=kT_sb[bp:bp + DK, et, jt * P:(jt + 1) * P],
                            rhs=qT_sb[bp:bp + DK, et, ic * F + col0:(ic + 1) * F],
                            start=True, stop=True,
                        )
                        pt = ptp.tile([P, F], F32R, tag="pt")
                        nc.scalar.activation(pt[:, col0:], ps[:, col0:], EXP)
                        if r >= 0:
                            nc.vector.tensor_tensor(
                                pt[:, col0:col0 + P], pt[:, col0:col0 + P],
                                tri_sb[:], MULT)
                        ptiles.append((pt, col0))
                    return ptiles

                def pv_stream(h, ic, ptiles):
                    """PV matmul with ones-augmented V; copy denom row to SBUF."""
                    njt = (ic + 1) * TPC
                    ps_o = pspv.tile([DK + 1, F], F32, tag="pspv")
                    for idx, (pt, col0) in enumerate(ptiles):
                        nc.tensor.matmul(
                            ps_o[:, col0:],
                            lhsT=v_sb[:, idx, h, :],
                            rhs=pt[:, col0:],
                            start=(idx == 0), stop=(idx == njt - 1),
                        )
                    den = denp.tile([DK + 1, F], F32R, tag="den")
                    nc.vector.tensor_copy(den[DK:DK + 1, :], ps_o[DK:DK + 1, :])
                    return ps_o, den

                def bc_norm(h, ic, ps_o, den):
                    """broadcast denom, invert, normalize into outT."""
                    et = h // 2
                    bp = (h % 2) * DK
                    ps_bc = psb.tile([DK, F], F32, tag="psb")
                    nc.tensor.matmul(
                        ps_bc,
                        lhsT=ones_sb[DK:DK + 1, :],
                        rhs=den[DK:DK + 1, :],
                        start=True, stop=True,
                    )
                    rcp = rcpp.tile([DK, F], F32, tag="rcp")
                    nc.vector.reciprocal_approx_fast(out=rcp[:], in_=ps_bc[:])
                    nc.vector.tensor_tensor(
                        outT_sb[bp:bp + DK, et, ic * F:(ic + 1) * F],
                        ps_o[:DK, :], rcp[:], MULT)

                for ic in range(NIC):
                    # head-pipelined: scores(h+1) before PV(h); bcast/norm(h)
                    # one more step behind.
                    pts = {}
                    pvs = {}
                    pts[0] = s_stream(0, ic)
                    pts[1] = s_stream(1, ic)
                    pvs[0] = pv_stream(0, ic, pts[0])
                    if ic >= 1:
                        emit_wo(ic - 1)
                    pts[2] = s_stream(2, ic)
                    bc_norm(0, ic, *pvs[0])
                    pvs[1] = pv_stream(1, ic, pts[1])
                    pts[3] = s_stream(3, ic)
                    bc_norm(1, ic, *pvs[1])
                    pvs[2] = pv_stream(2, ic, pts[2])
                    bc_norm(2, ic, *pvs[2])
                    pvs[3] = pv_stream(3, ic, pts[3])
                    bc_norm(3, ic, *pvs[3])
                emit_wo(NIC - 1)

    nc.compile()
    return nc


_CACHE = {}


def _get_nc(S):
    if S not in _CACHE:
        _CACHE[S] = build(S)
    return _CACHE[S]


def kernel(x, Wq, Wk, Wv, Wo):
    global LAST_RESULTS
    x = np.asarray(x, dtype=np.float32)
    Wq = np.asarray(Wq, dtype=np.float32)
    Wk = np.asarray(Wk, dtype=np.float32)
    Wv = np.asarray(Wv, dtype=np.float32)
    Wo = np.asarray(Wo, dtype=np.float32)
    B, S, D_ = x.shape
    nc = _get_nc(S)

    tri = np.triu(np.ones((P, P), np.float32))          # keep j' <= t
    ones = np.ones((P, DK), np.float32)
    scale = np.float32(1.0 / np.sqrt(DK))

    in_maps = []
    for c in range(8):
        b, g = divmod(c, 4)
        sl = slice(E * g, E * (g + 1))
        in_maps.append({
            "xT": np.ascontiguousarray(x[b].T),
            "wqT": np.ascontiguousarray((Wq[sl] * scale).T),
            "wkT": np.ascontiguousarray(Wk[sl].T),
            "wvT": np.ascontiguousarray(Wv[sl].T),
            "woT": np.ascontiguousarray(Wo[:, sl].T),
            "tri": tri,
            "ones": ones,
        })

    res = bass_utils.run_bass_kernel_spmd(
        nc, in_maps, core_ids=list(range(8)),
        trace=bool(os.environ.get("KERNEL_TRACE")),
    )
    LAST_RESULTS = res

    y = np.zeros((B, S, D_), np.float32)
    for c in range(8):
        y[c // 4] += res.results[c]["y"]
    return y


if __name__ == "__main__":
    # small-S self test against numpy
    S = 512
    rng = np.random.default_rng(0)
    B, H, dk = 2, 16, 64
    x = rng.standard_normal((B, S, D)).astype(np.float32)
    sc = 1.0 / np.sqrt(D)
    Wq = (rng.standard_normal((D, D)) * sc).astype(np.float32)
    Wk = (rng.standard_normal((D, D)) * sc).astype(np.float32)
    Wv = (rng.standard_normal((D, D)) * sc).astype(np.float32)
    Wo = (rng.standard_normal((D, D)) * sc).astype(np.float32)

    def ref(x, Wq, Wk, Wv, Wo):
        x64 = x.astype(np.float64)
        q = (x64 @ Wq.T.astype(np.float64)).reshape(B, S, H, dk).transpose(0, 2, 1, 3)
        k = (x64 @ Wk.T.astype(np.float64)).reshape(B, S, H, dk).transpose(0, 2, 1, 3)
        v = (x64 @ Wv.T.astype(np.float64)).reshape(B, S, H, dk).transpose(0, 2, 1, 3)
        s = np.einsum("bhid,bhjd->bhij", q, k) / np.sqrt(dk)
        mask = np.triu(np.ones((S, S), bool), k=1)
        s = np.where(mask, -np.inf, s)
        s -= s.max(axis=-1, keepdims=True)
        p = np.exp(s)
        p /= p.sum(axis=-1, keepdims=True)
        o = np.einsum("bhij,bhjd->bhid", p, v).transpose(0, 2, 1, 3).reshape(B, S, D)
        return o @ Wo.T.astype(np.float64)

    expected = ref(x, Wq, Wk, Wv, Wo)
    actual = kernel(x, Wq, Wk, Wv, Wo)
    err = np.abs(actual - expected).max() / np.abs(expected).max()
    print("self-test S=512 max rel err:", err)
    assert err < 2e-3, err
    print("PASS")


# revision 10
# speedup vs baseline: 1.3083x; 1.1781x over previous
"""Causal multi-head self-attention on 8 TRN2 NeuronCores (Bass/Tile).

Problem (hardcoded): x[2, 2048, 1024], Wq/Wk/Wv/Wo [1024, 1024] (nn.Linear
convention, out x in), H=16 heads, dk=64, causal softmax, y = attn @ Wo.T.

Sharding: 2-way data parallel (batch) x 4-way tensor parallel (head groups of
4). Each core computes q/k/v projections for its 4 heads, causal attention,
and a partial output projection against its 256-column slice of Wo. The host
sums the 4 partial [2048, 1024] outputs per batch (the "all-reduce").

Device kernel design notes:
  - Everything runs in "transposed" orientation so no on-device transposes are
    needed: QT/KT [256, S] = W @ x^T, V [S, 256] = x @ Wv^T, scoresT[j, i] per
    head, PV output [64+1, i], final y [i, o] (natural).
  - fp32r (TF32-like, 1 cycle/row for moving dim >= 256) for all matmuls;
    measured ~1e-4 matmul rel err.
  - Causal: tiles with j > i skipped entirely; diagonal-crossing [128, 512]
    tiles compute only columns >= 128*r and apply a [128, 128] triangular
    0/1 mask after exp. Softmax needs no max subtraction (|scores| <~ 7
    for this problem's N(0,1)-scale data; exp is safe in fp32).
  - Softmax denominator comes free from the PV matmul: V is augmented with a
    ones column (lhsT [j, 65]), so PSUM row 64 = sum_j p[j, i]. It is
    broadcast across 64 partitions with a K=1 outer-product matmul, inverted
    with one fast-reciprocal DVE op, and applied during the PV PSUM->SBUF
    move.
  - Emission is head-pipelined (scores of head h+1 are issued before PV of
    head h) so the tensor engine never waits on ScalarE's exp stream.
"""

import os
import numpy as np

import concourse.mybir as mybir
import concourse.tile as tile
from concourse import bacc
from concourse import bass_utils

F32 = mybir.dt.float32
F32R = mybir.dt.float32r
EXP = mybir.ActivationFunctionType.Exp
MULT = mybir.AluOpType.mult

P = 128        # partitions
F = 512        # free-dim chunk (fp32 max moving dim / one PSUM bank)
D = 1024       # model dim
E = 256        # per-core head-group width (4 heads x 64)
DK = 64        # head dim
HL = 4         # heads per core
NK = D // P    # contraction k-tiles for projections

LAST_RESULTS = None  # test harness can inspect exec_time_ns etc.


def build(S: int = 2048):
    """Build the per-core Bass program (same program on all 8 cores)."""
    NIC = S // F     # i-chunks
    NJT = S // P     # j-tiles
    TPC = F // P     # j-tiles per i-chunk (4)

    nc = bacc.Bacc("TRN2", target_bir_lowering=False, debug=False,
                   enable_asserts=False)
    xT_d = nc.dram_tensor("xT", [D, S], F32, kind="ExternalInput").ap()
    wqT_d = nc.dram_tensor("wqT", [D, E], F32, kind="ExternalInput").ap()
    wkT_d = nc.dram_tensor("wkT", [D, E], F32, kind="ExternalInput").ap()
    wvT_d = nc.dram_tensor("wvT", [D, E], F32, kind="ExternalInput").ap()
    woT_d = nc.dram_tensor("woT", [E, D], F32, kind="ExternalInput").ap()
    tri_d = nc.dram_tensor("tri", [P, P], F32, kind="ExternalInput").ap()
    ones_d = nc.dram_tensor("ones", [P, DK], F32, kind="ExternalInput").ap()
    y_d = nc.dram_tensor("y", [S, D], F32, kind="ExternalOutput").ap()

    with tile.TileContext(nc) as tc:
        with tc.tile_pool(name="persist", bufs=1) as pp:
            tri_sb = pp.tile([P, P], F32)
            nc.sync.dma_start(tri_sb[:], tri_d)
            ones_sb = pp.tile([P, DK], F32R)
            nc.sync.dma_start(ones_sb[:], ones_d.bitcast(F32R))
            wo_sb = pp.tile([P, E // P, D], F32R)
            qT_sb = pp.tile([P, E // P, S], F32R)
            kT_sb = pp.tile([P, E // P, S], F32R)
            v_sb = pp.tile([P, NJT, HL, DK + 1], F32R)
            outT_sb = pp.tile([P, E // P, S], F32R)

            # ---- Phase A: projections (QT, KT, V) ----
            with (
                tc.tile_pool(name="phA", bufs=1) as pa,
                tc.tile_pool(name="psA", bufs=8, space="PSUM") as psA,
            ):
                wq_sb = pa.tile([P, NK, E], F32R, tag="w")
                wk_sb = pa.tile([P, NK, E], F32R, tag="w")
                wv_sb = pa.tile([P, NK, E], F32R, tag="w")
                x_sb = pa.tile([P, NK, S], F32R)
                xT_r = xT_d.rearrange("(kt p) s -> p kt s", p=P).bitcast(F32R)
                wq_r = wqT_d.rearrange("(kt p) e -> p kt e", p=P).bitcast(F32R)
                wk_r = wkT_d.rearrange("(kt p) e -> p kt e", p=P).bitcast(F32R)
                # DMA priority: per-k weights + x chunks (QT/KT consume in k
                # order), then wv/wo (needed later).
                for k in range(NK):
                    nc.sync.dma_start(wq_sb[:, k], wq_r[:, k])
                    nc.sync.dma_start(wk_sb[:, k], wk_r[:, k])
                    nc.sync.dma_start(x_sb[:, k], xT_r[:, k])
                nc.sync.dma_start(wv_sb[:],
                                  wvT_d.rearrange("(kt p) e -> p kt e", p=P).bitcast(F32R))
                nc.sync.dma_start(wo_sb[:],
                                  woT_d.rearrange("(kt p) o -> p kt o", p=P).bitcast(F32R))

                # ones column of the augmented V (all j-tiles at once)
                nc.vector.tensor_copy(
                    v_sb[:, :, :, DK].rearrange("p a b -> p (a b)"),
                    ones_sb[:, :1].to_broadcast([P, NJT * HL]))

                # QT+KT k-outer across 8 concurrently-open PSUM groups so the
                # tensor engine tracks the x-chunk DMA arrivals instead of
                # stalling for the full xT.
                groups = [("q", et, ic) for et in range(E // P) for ic in range(NIC)] \
                       + [("k", et, ic) for et in range(E // P) for ic in range(NIC)]
                half = len(groups) // 2
                for gset in (groups[:half], groups[half:]):
                    pstiles = {g: psA.tile([P, F], F32, tag="psA", name=f"psA_{g[0]}{g[1]}{g[2]}") for g in gset}
                    for k in range(NK):
                        for g in gset:
                            which, et, ic = g
                            w = wq_sb if which == "q" else wk_sb
                            nc.tensor.matmul(
                                pstiles[g],
                                lhsT=w[:, k, et * P:(et + 1) * P],
                                rhs=x_sb[:, k, ic * F:(ic + 1) * F],
                                start=(k == 0), stop=(k == NK - 1),
                            )
                    for g in gset:
                        which, et, ic = g
                        dst = qT_sb if which == "q" else kT_sb
                        nc.any.tensor_copy(dst[:, et, ic * F:(ic + 1) * F],
                                           pstiles[g])
                # V: [S, E] = (xT k-tile).T @ wvT, accumulated over k
                for jt in range(NJT):
                    ps = psA.tile([P, F], F32, tag="psA")
                    for k in range(NK):
                        nc.tensor.matmul(
                            ps[:, :E],
                            lhsT=x_sb[:, k, jt * P:(jt + 1) * P],
                            rhs=wv_sb[:, k],
                            start=(k == 0), stop=(k == NK - 1),
                        )
                    nc.any.tensor_copy(
                        v_sb[:, jt, :, :DK],
                        ps[:, :E].rearrange("p (h d) -> p h d", h=HL))

            # ---- Phase B: attention + output projection ----
            with (
                tc.tile_pool(name="pT", bufs=min(8 * TPC + 6, 38)) as ptp,
                tc.tile_pool(name="den", bufs=3) as denp,
                tc.tile_pool(name="rcp", bufs=3) as rcpp,
                tc.tile_pool(name="ysb", bufs=4) as yp,
                tc.tile_pool(name="ps_s", bufs=3, space="PSUM") as pss,
                tc.tile_pool(name="ps_pv", bufs=3, space="PSUM") as pspv,
                tc.tile_pool(name="ps_by", bufs=2, space="PSUM") as psby,
            ):
                psb = psy = psby
                def emit_wo(ic):
                    # y[i-tile, :] = outT.T @ woT for the i-tiles of chunk ic
                    for it in range(ic * TPC, (ic + 1) * TPC):
                        for oc in range(D // F):
                            ps = psy.tile([P, F], F32, tag="psby")
                            for et in range(E // P):
                                nc.tensor.matmul(
                                    ps,
                                    lhsT=outT_sb[:, et, it * P:(it + 1) * P],
                                    rhs=wo_sb[:, et, oc * F:(oc + 1) * F],
                                    start=(et == 0), stop=(et == E // P - 1),
                                )
                            yt = yp.tile([P, F], F32, tag="y")
                            nc.vector.tensor_copy(yt[:], ps)
                            nc.sync.dma_start(
                                y_d[it * P:(it + 1) * P, oc * F:(oc + 1) * F], yt[:])

                def s_stream(h, ic):
                    """scores (transposed) + exp + causal mask for one head/chunk."""
                    et = h // 2
                    bp = (h % 2) * DK
                    njt = (ic + 1) * TPC
                    ptiles = []
                    for jt in range(njt):
                        r = jt - ic * TPC
                        col0 = max(0, r * P)
                        ps = pss.tile([P, F], F32, tag="pss")
                        nc.tensor.matmul(
                            ps[:, col0:],
                            lhsT=kT_sb[bp:bp + DK, et, jt * P:(jt + 1) * P],
                            rhs=qT_sb[bp:bp + DK, et, ic * F + col0:(ic + 1) * F],
                            start=True, stop=True,
                        )
                        pt = ptp.tile([P, F], F32R, tag="pt")
                        nc.scalar.activation(pt[:, col0:], ps[:, col0:], EXP)
                        if r >= 0:
                            nc.vector.tensor_tensor(
                                pt[:, col0:col0 + P], pt[:, col0:col0 + P],
                                tri_sb[:], MULT)
                        ptiles.append((pt, col0))
                    return ptiles

                def pv_stream(h, ic, ptiles):
                    """PV matmul with ones-augmented V; copy denom row to SBUF."""
                    njt = (ic + 1) * TPC
                    ps_o = pspv.tile([DK + 1, F], F32, tag="pspv")
                    for idx, (pt, col0) in enumerate(ptiles):
                        nc.tensor.matmul(
                            ps_o[:, col0:],
                            lhsT=v_sb[:, idx, h, :],
                            rhs=pt[:, col0:],
                            start=(idx == 0), stop=(idx == njt - 1),
                        )
                    den = denp.tile([DK + 1, F], F32R, tag="den")
                    nc.vector.tensor_copy(den[DK:DK + 1, :], ps_o[DK:DK + 1, :])
                    return ps_o, den

                def bc_norm(h, ic, ps_o, den):
                    """broadcast denom, invert, normalize into outT."""
                    et = h // 2
                    bp = (h % 2) * DK
                    ps_bc_full = psb.tile([P, F], F32, tag="psby", name="ps_bc")
                    ps_bc = ps_bc_full[:DK]
                    nc.tensor.matmul(
                        ps_bc,
                        lhsT=ones_sb[DK:DK + 1, :],
                        rhs=den[DK:DK + 1, :],
                        start=True, stop=True,
                    )
                    rcp = rcpp.tile([DK, F], F32, tag="rcp")
                    nc.vector.reciprocal_approx_fast(out=rcp[:], in_=ps_bc[:])
                    nc.vector.tensor_tensor(
                        outT_sb[bp:bp + DK, et, ic * F:(ic + 1) * F],
                        ps_o[:DK, :], rcp[:], MULT)

                for ic in range(NIC):
                    # head-pipelined: scores(h+1) before PV(h); bcast/norm(h)
                    # one more step behind.
                    pts = {}
                    pvs = {}
                    pts[0] = s_stream(0, ic)
                    pts[1] = s_stream(1, ic)
                    pvs[0] = pv_stream(0, ic, pts[0])
                    if ic >= 1:
                        emit_wo(ic - 1)
                    pts[2] = s_stream(2, ic)
                    bc_norm(0, ic, *pvs[0])
                    pvs[1] = pv_stream(1, ic, pts[1])
                    pts[3] = s_stream(3, ic)
                    bc_norm(1, ic, *pvs[1])
                    pvs[2] = pv_stream(2, ic, pts[2])
                    bc_norm(2, ic, *pvs[2])
                    pvs[3] = pv_stream(3, ic, pts[3])
                    bc_norm(3, ic, *pvs[3])
                emit_wo(NIC - 1)

    nc.compile()
    return nc


_CACHE = {}


def _get_nc(S):
    if S not in _CACHE:
        _CACHE[S] = build(S)
    return _CACHE[S]


def kernel(x, Wq, Wk, Wv, Wo):
    global LAST_RESULTS
    x = np.asarray(x, dtype=np.float32)
    Wq = np.asarray(Wq, dtype=np.float32)
    Wk = np.asarray(Wk, dtype=np.float32)
    Wv = np.asarray(Wv, dtype=np.float32)
    Wo = np.asarray(Wo, dtype=np.float32)
    B, S, D_ = x.shape
    nc = _get_nc(S)

    tri = np.triu(np.ones((P, P), np.float32))          # keep j' <= t
    ones = np.ones((P, DK), np.float32)
    scale = np.float32(1.0 / np.sqrt(DK))

    in_maps = []
    for c in range(8):
        b, g = divmod(c, 4)
        sl = slice(E * g, E * (g + 1))
        in_maps.append({
            "xT": np.ascontiguousarray(x[b].T),
            "wqT": np.ascontiguousarray((Wq[sl] * scale).T),
            "wkT": np.ascontiguousarray(Wk[sl].T),
            "wvT": np.ascontiguousarray(Wv[sl].T),
            "woT": np.ascontiguousarray(Wo[:, sl].T),
            "tri": tri,
            "ones": ones,
        })

    res = bass_utils.run_bass_kernel_spmd(
        nc, in_maps, core_ids=list(range(8)),
        trace=bool(os.environ.get("KERNEL_TRACE")),
    )
    LAST_RESULTS = res

    y = np.zeros((B, S, D_), np.float32)
    for c in range(8):
        y[c // 4] += res.results[c]["y"]
    return y


if __name__ == "__main__":
    # small-S self test against numpy
    S = 512
    rng = np.random.default_rng(0)
    B, H, dk = 2, 16, 64
    x = rng.standard_normal((B, S, D)).astype(np.float32)
    sc = 1.0 / np.sqrt(D)
    Wq = (rng.standard_normal((D, D)) * sc).astype(np.float32)
    Wk = (rng.standard_normal((D, D)) * sc).astype(np.float32)
    Wv = (rng.standard_normal((D, D)) * sc).astype(np.float32)
    Wo = (rng.standard_normal((D, D)) * sc).astype(np.float32)

    def ref(x, Wq, Wk, Wv, Wo):
        x64 = x.astype(np.float64)
        q = (x64 @ Wq.T.astype(np.float64)).reshape(B, S, H, dk).transpose(0, 2, 1, 3)
        k = (x64 @ Wk.T.astype(np.float64)).reshape(B, S, H, dk).transpose(0, 2, 1, 3)
        v = (x64 @ Wv.T.astype(np.float64)).reshape(B, S, H, dk).transpose(0, 2, 1, 3)
        s = np.einsum("bhid,bhjd->bhij", q, k) / np.sqrt(dk)
        mask = np.triu(np.ones((S, S), bool), k=1)
        s = np.where(mask, -np.inf, s)
        s -= s.max(axis=-1, keepdims=True)
        p = np.exp(s)
        p /= p.sum(axis=-1, keepdims=True)
        o = np.einsum("bhij,bhjd->bhid", p, v).transpose(0, 2, 1, 3).reshape(B, S, D)
        return o @ Wo.T.astype(np.float64)

    expected = ref(x, Wq, Wk, Wv, Wo)
    actual = kernel(x, Wq, Wk, Wv, Wo)
    err = np.abs(actual - expected).max() / np.abs(expected).max()
    print("self-test S=512 max rel err:", err)
    assert err < 2e-3, err
    print("PASS")


# revision 12
# speedup vs baseline: 1.3812x; 1.0558x over previous
"""Causal multi-head self-attention on 8 TRN2 NeuronCores (Bass/Tile).

Problem (hardcoded): x[2, 2048, 1024], Wq/Wk/Wv/Wo [1024, 1024] (nn.Linear
convention, out x in), H=16 heads, dk=64, causal softmax, y = attn @ Wo.T.

Sharding: 2-way data parallel (batch) x 4-way tensor parallel (head groups of
4). Each core computes q/k/v projections for its 4 heads, causal attention,
and a partial output projection against its 256-column slice of Wo. The host
sums the 4 partial [2048, 1024] outputs per batch (the "all-reduce").

Device kernel design notes:
  - Everything runs in "transposed" orientation so no on-device transposes are
    needed: QT/KT [256, S] = W @ x^T, V [S, 256] = x @ Wv^T, scoresT[j, i] per
    head, PV output [64+1, i], final y [i, o] (natural).
  - fp32r (TF32-like, 1 cycle/row for moving dim >= 256) for all matmuls;
    measured ~1e-4 matmul rel err.
  - Causal: tiles with j > i skipped entirely; diagonal-crossing [128, 512]
    tiles compute only columns >= 128*r and apply a [128, 128] triangular
    0/1 mask after exp. Softmax needs no max subtraction (|scores| <~ 7
    for this problem's N(0,1)-scale data; exp is safe in fp32).
  - Softmax denominator comes free from the PV matmul: V is augmented with a
    ones column (lhsT [j, 65]), so PSUM row 64 = sum_j p[j, i]. It is
    broadcast across 64 partitions with a K=1 outer-product matmul, inverted
    with one fast-reciprocal DVE op, and applied during the PV PSUM->SBUF
    move.
  - Emission is head-pipelined (scores of head h+1 are issued before PV of
    head h) so the tensor engine never waits on ScalarE's exp stream.
"""

import os
import numpy as np

import concourse.mybir as mybir
import concourse.tile as tile
from concourse import bacc
from concourse import bass_utils

F32 = mybir.dt.float32
F32R = mybir.dt.float32r
EXP = mybir.ActivationFunctionType.Exp
MULT = mybir.AluOpType.mult

P = 128        # partitions
F = 512        # free-dim chunk (fp32 max moving dim / one PSUM bank)
D = 1024       # model dim
E = 256        # per-core head-group width (4 heads x 64)
DK = 64        # head dim
HL = 4         # heads per core
NK = D // P    # contraction k-tiles for projections

LAST_RESULTS = None  # test harness can inspect exec_time_ns etc.


def build(S: int = 2048):
    """Build the per-core Bass program (same program on all 8 cores)."""
    NIC = S // F     # i-chunks
    NJT = S // P     # j-tiles
    TPC = F // P     # j-tiles per i-chunk (4)

    nc = bacc.Bacc("TRN2", target_bir_lowering=False, debug=False,
                   enable_asserts=False)
    xT_d = nc.dram_tensor("xT", [D, S], F32, kind="ExternalInput").ap()
    wqT_d = nc.dram_tensor("wqT", [D, E], F32, kind="ExternalInput").ap()
    wkT_d = nc.dram_tensor("wkT", [D, E], F32, kind="ExternalInput").ap()
    wvT_d = nc.dram_tensor("wvT", [D, E], F32, kind="ExternalInput").ap()
    woT_d = nc.dram_tensor("woT", [E, D], F32, kind="ExternalInput").ap()
    tri_d = nc.dram_tensor("tri", [P, P], F32, kind="ExternalInput").ap()
    ones_d = nc.dram_tensor("ones", [P, DK], F32, kind="ExternalInput").ap()
    y_d = nc.dram_tensor("y", [S, D], F32, kind="ExternalOutput").ap()

    with tile.TileContext(nc) as tc:
        with tc.tile_pool(name="persist", bufs=1) as pp:
            tri_sb = pp.tile([P, P], F32)
            nc.sync.dma_start(tri_sb[:], tri_d)
            ones_sb = pp.tile([P, DK], F32R)
            nc.sync.dma_start(ones_sb[:], ones_d.bitcast(F32R))
            wo_sb = pp.tile([P, E // P, D], F32R)
            qT_sb = pp.tile([P, E // P, S], F32R)
            kT_sb = pp.tile([P, E // P, S], F32R)
            v_sb = pp.tile([P, NJT, HL, DK + 1], F32R)
            outT_sb = pp.tile([P, E // P, S], F32R)

            # ---- Phase A: projections (QT, KT, V) ----
            with (
                tc.tile_pool(name="phA", bufs=1) as pa,
                tc.tile_pool(name="psA", bufs=8, space="PSUM") as psA,
            ):
                wq_sb = pa.tile([P, NK, E], F32R)
                wk_sb = pa.tile([P, NK, E], F32R)
                wv_sb = pa.tile([P, NK, E], F32R)
                x_sb = pa.tile([P, NK, S], F32R)
                xT_r = xT_d.rearrange("(kt p) s -> p kt s", p=P).bitcast(F32R)
                wq_r = wqT_d.rearrange("(kt p) e -> p kt e", p=P).bitcast(F32R)
                wk_r = wkT_d.rearrange("(kt p) e -> p kt e", p=P).bitcast(F32R)
                # DMA priority: per-k weights + x chunks (QT/KT consume in k
                # order); wv/wo/tri/ones slotted mid-stream (needed later).
                for k in range(NK):
                    nc.sync.dma_start(wq_sb[:, k], wq_r[:, k])
                    nc.sync.dma_start(wk_sb[:, k], wk_r[:, k])
                    nc.sync.dma_start(x_sb[:, k], xT_r[:, k])
                    if k == 3:
                        nc.sync.dma_start(
                            wv_sb[:],
                            wvT_d.rearrange("(kt p) e -> p kt e", p=P).bitcast(F32R))
                    if k == 4:
                        nc.sync.dma_start(
                            wo_sb[:],
                            woT_d.rearrange("(kt p) o -> p kt o", p=P).bitcast(F32R))

                # ones column of the augmented V (all j-tiles at once)
                nc.vector.tensor_copy(
                    v_sb[:, :, :, DK].rearrange("p a b -> p (a b)"),
                    ones_sb[:, :1].to_broadcast([P, NJT * HL]))

                # QT+KT k-outer across 8 concurrently-open PSUM groups so the
                # tensor engine tracks the x-chunk DMA arrivals instead of
                # stalling for the full xT.
                groups = [("q", et, ic) for et in range(E // P) for ic in range(NIC)] \
                       + [("k", et, ic) for et in range(E // P) for ic in range(NIC)]
                half = len(groups) // 2
                for gset in (groups[:half], groups[half:]):
                    pstiles = {g: psA.tile([P, F], F32, tag="psA", name=f"psA_{g[0]}{g[1]}{g[2]}") for g in gset}
                    for k in range(NK):
                        for g in gset:
                            which, et, ic = g
                            w = wq_sb if which == "q" else wk_sb
                            nc.tensor.matmul(
                                pstiles[g],
                                lhsT=w[:, k, et * P:(et + 1) * P],
                                rhs=x_sb[:, k, ic * F:(ic + 1) * F],
                                start=(k == 0), stop=(k == NK - 1),
                            )
                    for g in gset:
                        which, et, ic = g
                        dst = qT_sb if which == "q" else kT_sb
                        nc.any.tensor_copy(dst[:, et, ic * F:(ic + 1) * F],
                                           pstiles[g])
                # V: [S, E] = (xT k-tile).T @ wvT, k-outer in two 8-group sets
                for jset in (range(0, NJT // 2), range(NJT // 2, NJT)):
                    vtiles = {jt: psA.tile([P, F], F32, tag="psA",
                                           name=f"psV_{jt}") for jt in jset}
                    for k in range(NK):
                        for jt in jset:
                            nc.tensor.matmul(
                                vtiles[jt][:, :E],
                                lhsT=x_sb[:, k, jt * P:(jt + 1) * P],
                                rhs=wv_sb[:, k],
                                start=(k == 0), stop=(k == NK - 1),
                            )
                    for jt in jset:
                        nc.any.tensor_copy(
                            v_sb[:, jt, :, :DK],
                            vtiles[jt][:, :E].rearrange("p (h d) -> p h d", h=HL))

            # ---- Phase B: attention + output projection ----
            with (
                tc.tile_pool(name="pT", bufs=min(8 * TPC + 6, 38)) as ptp,
                tc.tile_pool(name="den", bufs=3) as denp,
                tc.tile_pool(name="rcp", bufs=3) as rcpp,
                tc.tile_pool(name="ysb", bufs=4) as yp,
                tc.tile_pool(name="ps_s", bufs=3, space="PSUM") as pss,
                tc.tile_pool(name="ps_pv", bufs=3, space="PSUM") as pspv,
                tc.tile_pool(name="ps_by", bufs=2, space="PSUM") as psby,
            ):
                psb = psy = psby
                def emit_wo(ic):
                    # y[i-tile, :] = outT.T @ woT for the i-tiles of chunk ic
                    for it in range(ic * TPC, (ic + 1) * TPC):
                        for oc in range(D // F):
                            ps = psy.tile([P, F], F32, tag="psby")
                            for et in range(E // P):
                                nc.tensor.matmul(
                                    ps,
                                    lhsT=outT_sb[:, et, it * P:(it + 1) * P],
                                    rhs=wo_sb[:, et, oc * F:(oc + 1) * F],
                                    start=(et == 0), stop=(et == E // P - 1),
                                )
                            yt = yp.tile([P, F], F32, tag="y")
                            nc.vector.tensor_copy(yt[:], ps)
                            nc.sync.dma_start(
                                y_d[it * P:(it + 1) * P, oc * F:(oc + 1) * F], yt[:])

                def s_stream(h, ic):
                    """scores (transposed) + exp + causal mask for one head/chunk."""
                    et = h // 2
                    bp = (h % 2) * DK
                    njt = (ic + 1) * TPC
                    ptiles = []
                    for jt in range(njt):
                        r = jt - ic * TPC
                        col0 = max(0, r * P)
                        ps = pss.tile([P, F], F32, tag="pss")
                        nc.tensor.matmul(
                            ps[:, col0:],
                            lhsT=kT_sb[bp:bp + DK, et, jt * P:(jt + 1) * P],
                            rhs=qT_sb[bp:bp + DK, et, ic * F + col0:(ic + 1) * F],
                            start=True, stop=True,
                        )
                        pt = ptp.tile([P, F], F32R, tag="pt")
                        nc.scalar.activation(pt[:, col0:], ps[:, col0:], EXP)
                        if r >= 0:
                            nc.vector.tensor_tensor(
                                pt[:, col0:col0 + P], pt[:, col0:col0 + P],
                                tri_sb[:], MULT)
                        ptiles.append((pt, col0))
                    return ptiles

                def pv_stream(h, ic, ptiles):
                    """PV matmul with ones-augmented V; copy denom row to SBUF."""
                    njt = (ic + 1) * TPC
                    ps_o = pspv.tile([DK + 1, F], F32, tag="pspv")
                    for idx, (pt, col0) in enumerate(ptiles):
                        nc.tensor.matmul(
                            ps_o[:, col0:],
                            lhsT=v_sb[:, idx, h, :],
                            rhs=pt[:, col0:],
                            start=(idx == 0), stop=(idx == njt - 1),
                        )
                    den = denp.tile([DK + 1, F], F32R, tag="den")
                    nc.vector.tensor_copy(den[DK:DK + 1, :], ps_o[DK:DK + 1, :])
                    return ps_o, den

                def bc_norm(h, ic, ps_o, den):
                    """broadcast denom, invert, normalize into outT."""
                    et = h // 2
                    bp = (h % 2) * DK
                    ps_bc_full = psb.tile([P, F], F32, tag="psby", name="ps_bc")
                    ps_bc = ps_bc_full[:DK]
                    nc.tensor.matmul(
                        ps_bc,
                        lhsT=ones_sb[DK:DK + 1, :],
                        rhs=den[DK:DK + 1, :],
                        start=True, stop=True,
                    )
                    rcp = rcpp.tile([DK, F], F32, tag="rcp")
                    nc.vector.reciprocal_approx_fast(out=rcp[:], in_=ps_bc[:])
                    nc.vector.tensor_tensor(
                        outT_sb[bp:bp + DK, et, ic * F:(ic + 1) * F],
                        ps_o[:DK, :], rcp[:], MULT)

                for ic in range(NIC):
                    # head-pipelined: scores(h+1) before PV(h); bcast/norm(h)
                    # one more step behind.
                    pts = {}
                    pvs = {}
                    pts[0] = s_stream(0, ic)
                    pts[1] = s_stream(1, ic)
                    pvs[0] = pv_stream(0, ic, pts[0])
                    if ic >= 1:
                        emit_wo(ic - 1)
                    pts[2] = s_stream(2, ic)
                    bc_norm(0, ic, *pvs[0])
                    pvs[1] = pv_stream(1, ic, pts[1])
                    pts[3] = s_stream(3, ic)
                    bc_norm(1, ic, *pvs[1])
                    pvs[2] = pv_stream(2, ic, pts[2])
                    bc_norm(2, ic, *pvs[2])
                    pvs[3] = pv_stream(3, ic, pts[3])
                    bc_norm(3, ic, *pvs[3])
                emit_wo(NIC - 1)

    nc.compile()
    return nc


_CACHE = {}


def _get_nc(S):
    if S not in _CACHE:
        _CACHE[S] = build(S)
    return _CACHE[S]


def kernel(x, Wq, Wk, Wv, Wo):
    global LAST_RESULTS
    x = np.asarray(x, dtype=np.float32)
    Wq = np.asarray(Wq, dtype=np.float32)
    Wk = np.asarray(Wk, dtype=np.float32)
    Wv = np.asarray(Wv, dtype=np.float32)
    Wo = np.asarray(Wo, dtype=np.float32)
    B, S, D_ = x.shape
    nc = _get_nc(S)

    tri = np.triu(np.ones((P, P), np.float32))          # keep j' <= t
    ones = np.ones((P, DK), np.float32)
    scale = np.float32(1.0 / np.sqrt(DK))

    in_maps = []
    for c in range(8):
        b, g = divmod(c, 4)
        sl = slice(E * g, E * (g + 1))
        in_maps.append({
            "xT": np.ascontiguousarray(x[b].T),
            "wqT": np.ascontiguousarray((Wq[sl] * scale).T),
            "wkT": np.ascontiguousarray(Wk[sl].T),
            "wvT": np.ascontiguousarray(Wv[sl].T),
            "woT": np.ascontiguousarray(Wo[:, sl].T),
            "tri": tri,
            "ones": ones,
        })

    res = bass_utils.run_bass_kernel_spmd(
        nc, in_maps, core_ids=list(range(8)),
        trace=bool(os.environ.get("KERNEL_TRACE")),
    )
    LAST_RESULTS = res

    y = np.zeros((B, S, D_), np.float32)
    for c in range(8):
        y[c // 4] += res.results[c]["y"]
    return y


if __name__ == "__main__":
    # small-S self test against numpy
    S = 512
    rng = np.random.default_rng(0)
    B, H, dk = 2, 16, 64
    x = rng.standard_normal((B, S, D)).astype(np.float32)
    sc = 1.0 / np.sqrt(D)
    Wq = (rng.standard_normal((D, D)) * sc).astype(np.float32)
    Wk = (rng.standard_normal((D, D)) * sc).astype(np.float32)
    Wv = (rng.standard_normal((D, D)) * sc).astype(np.float32)
    Wo = (rng.standard_normal((D, D)) * sc).astype(np.float32)

    def ref(x, Wq, Wk, Wv, Wo):
        x64 = x.astype(np.float64)
        q = (x64 @ Wq.T.astype(np.float64)).reshape(B, S, H, dk).transpose(0, 2, 1, 3)
        k = (x64 @ Wk.T.astype(np.float64)).reshape(B, S, H, dk).transpose(0, 2, 1, 3)
        v = (x64 @ Wv.T.astype(np.float64)).reshape(B, S, H, dk).transpose(0, 2, 1, 3)
        s = np.einsum("bhid,bhjd->bhij", q, k) / np.sqrt(dk)
        mask = np.triu(np.ones((S, S), bool), k=1)
        s = np.where(mask, -np.inf, s)
        s -= s.max(axis=-1, keepdims=True)
        p = np.exp(s)
        p /= p.sum(axis=-1, keepdims=True)
        o = np.einsum("bhij,bhjd->bhid", p, v).transpose(0, 2, 1, 3).reshape(B, S, D)
        return o @ Wo.T.astype(np.float64)

    expected = ref(x, Wq, Wk, Wv, Wo)
    actual = kernel(x, Wq, Wk, Wv, Wo)
    err = np.abs(actual - expected).max() / np.abs(expected).max()
    print("self-test S=512 max rel err:", err)
    assert err < 2e-3, err
    print("PASS")
